# revision 10
# baseline (speedup 1.0000x reference)
"""GED layer (nn_GedLayer) on Trainium2 via Bass/Tile.

The reference builds an 8281x8281 cost matrix C = F + diag(c) where every
term of F is a Kronecker product of 91x91 matrices, runs a 10-iteration
Sinkhorn projection, and evaluates ged = 0.5*v'(F - diag(F))v + c'v.

Nothing 8281-sized is ever materialized here:
  * Sinkhorn on S = exp(-0.5*Dg) only rescales rows/cols, so we track the
    cumulative scale vectors:  R <- 1/(S0 @ C) (rows 0..89),
    C <- 1/(S0' @ R) (cols 0..89) -- two 91x91 matvecs per iteration on PE.
  * kron(P,Q) contractions reduce to 91x91 matmuls:
      sum V.(L V Rt') for (L,Rt) in {(B1,B2),(G1',H2),(G2',H1)}
    with V = diag(R) S0 diag(C), plus two rank-1 terms and the diagonal
    correction -- fused into per-partition accumulations + one ones-matvec.

Engine split: PE does matvecs/matmuls (fp16 operands, fp32 PSUM, 128-col
stationaries for fast weight load), DVE does the Sinkhorn reciprocals and
multiply-accumulate reductions, ACT does exp and all diag() row-scales
(activation Copy with a per-partition scale vector). Accuracy vs the fp32
reference: ~5e-4 rel, tolerance 2e-2. The host only decodes the int
adjacency/labels into the packed fp16 operand matrix (layout prep) and
shards it to the 8 cores; the problem is a single graph pair, so the
cores run the same tiny SPMD program and core 0's scalar is returned.
"""

import numpy as np

import concourse.bass as bass
import concourse.bacc as bacc
import concourse.mybir as mybir
import concourse.tile as tile
from concourse.bass_utils import run_bass_kernel_spmd

N = 91          # n+1 == m+1 with the epsilon node
NN = 90
P = 128         # padded stationary width (fast weight load needs 128 cols)
SINKHORN_ITERS = 10
NB_LABELS = 10
F16 = mybir.dt.float16
F32 = mybir.dt.float32
N_CORES = 8
COPY = mybir.ActivationFunctionType.Copy
EXP = mybir.ActivationFunctionType.Exp

# fp16 block layout: (name -> (col offset, width)). Stationary operands
# padded to 128 cols; pad cols of Dg/DgT hold 80.0 so exp(-0.5x) pads to 0.
_B16 = {}
_off = 0
for _name, _w in [("Dgp", P), ("DgTp", P), ("HB1p", P), ("HB2Tp", P),
                  ("HG1p", P), ("HG2p", P), ("HH2T", N), ("HH1T", N),
                  ("I91", N), ("DIAGF", N), ("MISC", 8)]:
    _B16[_name] = (_off, _w)
    _off += _w
_N16 = _off  # 1140


def _b16(name, w=None):
    o, bw = _B16[name]
    return slice(o, o + (w or bw))


class _FastTailTileContext(tile.TileContext):
    """TileContext with a single-shot epilogue: the NEFF runs exactly once
    per load here, so skip the semaphore/DMA-ring recycling and the second
    all-engine butterfly -- keep only the global drain (output-DMA safety)
    and a sequencer-level barrier."""

    def _drain_and_barrier(self, tick_clock, wait_clock):
        drain_inst = self.nc.sync.drain()
        wait_clock.add_sem_waits(
            drain_inst.ins, tile.ScopedClock({None: tick_clock.global_clock}))
        self.nc.all_engine_barrier(sem_only=True)
        popped = self.nc._tile_sem_poison_stack.pop()
        assert popped is self._sem_poison


def build_nc() -> bacc.Bacc:
    nc = bacc.Bacc(None, target_bir_lowering=False, debug=False)
    hostf16 = nc.dram_tensor("hostf16", [N, _N16], F16, kind="ExternalInput")
    out = nc.dram_tensor("out", [1, 1], F32, kind="ExternalOutput")
    mult = mybir.AluOpType.mult

    with _FastTailTileContext(nc) as tc:
        with (
            tc.tile_pool(name="persist", bufs=1) as pp,
            tc.tile_pool(name="rot", bufs=3) as rot,
            tc.tile_pool(name="psv", bufs=3, space=bass.MemorySpace.PSUM) as psv,
            tc.tile_pool(name="psm", bufs=3, space=bass.MemorySpace.PSUM) as psm,
            tc.tile_pool(name="psr", bufs=1, space=bass.MemorySpace.PSUM) as psr,
        ):
            zb = pp.tile([N, 1], F32)
            nc.vector.memset(zb[:], 0.0)
            warm = pp.tile([1, 1], F32)
            # prewarm the ACT function table during the DMA loads
            nc.scalar.activation(warm[:], zb[0:1, :], EXP,
                                 bias=zb[0:1, :], scale=1.0)

            hm = pp.tile([N, _N16], F16)
            # Dgp/DgTp land first so exp + Sinkhorn start early
            nc.sync.dma_start(hm[:, 0:2 * P], hostf16[:, 0:2 * P])
            nc.sync.dma_start(hm[:, 2 * P:_N16], hostf16[:, 2 * P:_N16])
            mo = _B16["MISC"][0]
            wv = hm[0:1, mo:mo + 7]

            Rb = pp.tile([N, 1], F16)
            nc.vector.memset(Rb[:], 1.0)
            Cb = pp.tile([N, 1], F16)
            nc.vector.memset(Cb[:], 1.0)

            # [S0 | S0T] = exp(-0.5 * [Dgp | DgTp]), fp16, pad cols -> 0
            eh = pp.tile([N, 2 * P], F16)
            nc.scalar.activation(eh[:], hm[:, 0:2 * P], EXP,
                                 bias=zb[:], scale=-0.5)
            s0 = eh[:, 0:P]
            s0t = eh[:, P:2 * P]

            # ---- Sinkhorn: R <- 1/(S0@C)[0:90], C <- 1/(S0'@R)[0:90] ----
            u = w = None
            with nc.allow_low_precision("fp16 pipeline; tol 2e-2"):
                for _ in range(SINKHORN_ITERS):
                    u = psv.tile([P, 1], F32, tag="vec")
                    nc.tensor.matmul(u[:], s0t, Cb[:])       # u = S0 @ C
                    nc.vector.reciprocal(Rb[0:NN, :], u[0:NN, :])
                    w = psv.tile([P, 1], F32, tag="vec")
                    nc.tensor.matmul(w[:], s0, Rb[:])        # w = S0' @ R
                    nc.vector.reciprocal(Cb[0:NN, :], w[0:NN, :])

                # fp32 copies of the final scales (off the critical path)
                Rf = pp.tile([N, 1], F32)
                nc.vector.memset(Rf[:], 1.0)
                nc.vector.reciprocal(Rf[0:NN, :], u[0:NN, :])
                Cf = pp.tile([N, 1], F32)
                nc.vector.memset(Cf[:], 1.0)
                nc.vector.reciprocal(Cf[0:NN, :], w[0:NN, :])

                cols = pp.tile([N, 7], F16)

                # T2: q0 = Vcol1 = C o w ; q = B2 @ q0 ; col1 = q0 o q
                q0 = pp.tile([N, 1], F16)
                nc.scalar.activation(q0[:], w[0:N, :], COPY, scale=Cf[:])
                q = psv.tile([P, 1], F32, tag="vec")
                nc.tensor.matmul(q[:], hm[:, _b16("HB2Tp")], q0[:])
                nc.vector.tensor_tensor(cols[:, 1:2], q[0:N, :], q0[:], mult)

                # T1: a = Vrow1 = R o (S0@C) ; p = B1 @ a ; col0 = a o p
                u2 = psv.tile([P, 1], F32, tag="vec")
                nc.tensor.matmul(u2[:], s0t, Cb[:])
                a = pp.tile([N, 1], F16)
                nc.scalar.activation(a[:], u2[0:N, :], COPY, scale=Rf[:])
                p = psv.tile([P, 1], F32, tag="vec")
                nc.tensor.matmul(p[:], hm[:, _b16("HB1p")], a[:])
                nc.vector.tensor_tensor(cols[:, 0:1], p[0:N, :], a[:], mult)

                # V = diag(R) S0 diag(C)   (via Bt = I)
                ic = rot.tile([N, N], F16, tag="bts")
                nc.scalar.activation(ic[:], hm[:, _b16("I91")], COPY,
                                     scale=Cf[:])
                pv = psm.tile([P, N], F32, tag="mat")
                nc.tensor.matmul(pv[:], s0t, ic[:])
                V = pp.tile([N, N], F32)
                nc.scalar.activation(V[:], pv[0:N, :], COPY, scale=Rf[:])

                # pair terms: col(2+i) = sum_k V o (L V Bt')
                pairs = [("HB1p", "HB2Tp"), ("HG1p", "HH2T"), ("HG2p", "HH1T")]
                for i, (lh, bt) in enumerate(pairs):
                    bo = _B16[bt][0]
                    bts = rot.tile([N, N], F16, tag="bts")
                    nc.scalar.activation(bts[:], hm[:, bo:bo + N], COPY,
                                         scale=Cf[:])
                    p1 = psm.tile([P, N], F32, tag="mat")
                    nc.tensor.matmul(p1[:], s0t, bts[:])   # S0 @ (diag(C) Bt)
                    y = rot.tile([N, N], F16, tag="y")
                    nc.scalar.activation(y[:], p1[0:N, :], COPY, scale=Rf[:])
                    z = psm.tile([P, N], F32, tag="mat")
                    nc.tensor.matmul(z[:], hm[:, _b16(lh)], y[:])  # L @ Y
                    jk = rot.tile([N, N], F32, tag="jk")
                    nc.vector.scalar_tensor_tensor(
                        jk[:], V[:], 1.0, z[0:N, :], op0=mult, op1=mult,
                        accum_out=cols[:, 2 + i:3 + i])

                # diag correction: col5 = sum_k diagF o V o V
                d2 = rot.tile([N, N], F32, tag="d2")
                nc.vector.tensor_tensor(d2[:], hm[:, _b16("DIAGF")], V[:],
                                        mult)
                jk = rot.tile([N, N], F32, tag="jk")
                nc.vector.scalar_tensor_tensor(
                    jk[:], d2[:], 1.0, V[:], op0=mult, op1=mult,
                    accum_out=cols[:, 5:6])
                # linear term: col6 = sum_k Dg o V
                jk = rot.tile([N, N], F32, tag="jk")
                nc.vector.scalar_tensor_tensor(
                    jk[:], hm[:, 0:N], 1.0, V[:], op0=mult, op1=mult,
                    accum_out=cols[:, 6:7])

                # fold partitions, weight, reduce to the ged scalar
                onesc = pp.tile([N, 1], F16)
                nc.vector.memset(onesc[:], 1.0)
                row = psr.tile([1, 7], F32)
                nc.tensor.matmul(row[:], onesc[:], cols[:])
                jr = pp.tile([1, 7], F32)
                g = pp.tile([1, 1], F32)
                nc.vector.scalar_tensor_tensor(
                    jr[:], row[:], 1.0, wv, op0=mult, op1=mult,
                    accum_out=g[:])
            nc.sync.dma_start(out[:], g[:])
    nc.compile()
    return nc


def host_prep(inputs: dict) -> np.ndarray:
    """Decode int adjacency/labels into the packed fp16 operand matrix."""
    node_weighs = np.asarray(inputs["node_weighs"], np.float32).reshape(-1)
    edge_weighs = np.asarray(inputs["edge_weighs"], np.float32).reshape(-1)
    cn = np.maximum(node_weighs, 0.0)
    ce = np.maximum(edge_weighs, 0.0)
    iu, ju = np.triu_indices(NB_LABELS, k=1)
    node_costs = np.zeros((NB_LABELS, NB_LABELS), np.float32)
    node_costs[iu, ju] = cn[:-1]
    node_costs = node_costs + node_costs.T
    nodeInsDel = np.float32(cn[-1])
    e = np.float32(ce[0])
    beta = np.float32(ce[-1])

    A = np.asarray(inputs["adjacenceMatrix"])
    A1 = np.zeros((N, N), np.int64)
    A1[:NN, :NN] = np.asarray(A[0][: NN * NN], np.int64).reshape(NN, NN)
    A2 = np.zeros((N, N), np.int64)
    A2[:NN, :NN] = np.asarray(A[1][: NN * NN], np.int64).reshape(NN, NN)
    Abin1 = (A1 != 0).astype(np.float16)
    Abin2 = (A2 != 0).astype(np.float16)
    G1 = (A1 == 1).astype(np.float16)
    G2 = (A1 == 2).astype(np.float16)
    H1 = (A2 == 1).astype(np.float16)
    H2 = (A2 == 2).astype(np.float16)

    labels = np.asarray(inputs["labels"])
    l1 = np.asarray(labels[0][:NN], np.int64)
    l2 = np.asarray(labels[1][:NN], np.int64)
    Dg = np.zeros((N, N), np.float32)
    Dg[:NN, :NN] = node_costs[l1[:, None], l2[None, :]]
    Dg[:NN, NN] = nodeInsDel
    Dg[NN, :NN] = nodeInsDel

    d1 = np.diag(Abin1).astype(np.float32)
    d2 = np.diag(Abin2).astype(np.float32)
    diagF = beta * (d1[:, None] + d2[None, :] - 2.0 * np.outer(d1, d2))
    diagF = diagF + e * (np.outer(np.diag(G1), np.diag(H2)).astype(np.float32)
                         + np.outer(np.diag(G2), np.diag(H1)).astype(np.float32))

    hostf16 = np.zeros((N, _N16), np.float16)

    def put(name, mat, pad=0.0):
        o, wd = _B16[name]
        hostf16[:, o:o + wd] = pad
        hostf16[:, o:o + mat.shape[1]] = mat

    put("Dgp", Dg.astype(np.float16), pad=80.0)      # exp(-0.5*80) -> 0
    put("DgTp", Dg.T.astype(np.float16), pad=80.0)
    put("HB1p", Abin1)
    put("HB2Tp", Abin2.T)
    put("HG1p", G1)
    put("HG2p", G2)
    put("HH2T", H2.T)
    put("HH1T", H1.T)
    put("I91", np.eye(N, dtype=np.float16))
    put("DIAGF", diagF.astype(np.float16))
    mo = _B16["MISC"][0]
    hostf16[0, mo:mo + 7] = np.array(
        [0.5 * beta, 0.5 * beta, -beta, 0.5 * e, 0.5 * e, -0.5, 1.0],
        np.float16)
    return hostf16


_NC_CACHE: list = []


def _run(inputs: dict, trace: bool = False):
    hostf16 = host_prep(inputs)
    if not _NC_CACHE:
        _NC_CACHE.append(build_nc())
    nc = _NC_CACHE[0]
    res = run_bass_kernel_spmd(
        nc, [{"hostf16": hostf16} for _ in range(N_CORES)],
        core_ids=list(range(N_CORES)), trace=trace)
    val = np.float32(np.asarray(res.results[0]["out"]).reshape(-1)[0])
    return val, res


def kernel(graph, adjacenceMatrix, graphCard, labels, node_weighs, edge_weighs):
    val, _ = _run({
        "adjacenceMatrix": adjacenceMatrix, "labels": labels,
        "node_weighs": node_weighs, "edge_weighs": edge_weighs,
    })
    return val


# revision 11
# speedup vs baseline: 1.0510x; 1.0510x over previous
"""GED layer (nn_GedLayer) on Trainium2 via Bass/Tile.

The reference builds an 8281x8281 cost matrix C = F + diag(c) where every
term of F is a Kronecker product of 91x91 matrices, runs a 10-iteration
Sinkhorn projection, and evaluates ged = 0.5*v'(F - diag(F))v + c'v.

Nothing 8281-sized is ever materialized here:
  * Sinkhorn on S = exp(-0.5*Dg) only rescales rows/cols, so we track the
    cumulative scale vectors:  R <- 1/(S0 @ C) (rows 0..89),
    C <- 1/(S0' @ R) (cols 0..89) -- two 91x91 matvecs per iteration on PE.
  * kron(P,Q) contractions reduce to 91x91 matmuls:
      sum V.(L V Rt') for (L,Rt) in {(B1,B2),(G1',H2),(G2',H1)}
    with V = diag(R) S0 diag(C), plus two rank-1 terms and the diagonal
    correction -- fused into per-partition accumulations + one ones-matvec.

Engine split: PE does matvecs/matmuls (fp16 operands, fp32 PSUM, 128-col
stationaries for fast weight load), DVE does the Sinkhorn reciprocals and
multiply-accumulate reductions, ACT does exp and all diag() row-scales
(activation Copy with a per-partition scale vector). Accuracy vs the fp32
reference: ~5e-4 rel, tolerance 2e-2. The host only decodes the int
adjacency/labels into the packed fp16 operand matrix (layout prep) and
shards it to the 8 cores; the problem is a single graph pair, so the
cores run the same tiny SPMD program and core 0's scalar is returned.
"""

import numpy as np

import concourse.bass as bass
import concourse.bacc as bacc
import concourse.mybir as mybir
import concourse.tile as tile
from concourse.bass_utils import run_bass_kernel_spmd

N = 91          # n+1 == m+1 with the epsilon node
NN = 90
P = 128         # padded stationary width (fast weight load needs 128 cols)
SINKHORN_ITERS = 10
NB_LABELS = 10
F16 = mybir.dt.float16
F32 = mybir.dt.float32
N_CORES = 8
COPY = mybir.ActivationFunctionType.Copy
EXP = mybir.ActivationFunctionType.Exp

# fp16 block layout: (name -> (col offset, width)). Stationary operands
# padded to 128 cols; pad cols of Dg/DgT hold 80.0 so exp(-0.5x) pads to 0.
_B16 = {}
_off = 0
for _name, _w in [("Dgp", P), ("DgTp", P), ("HB1p", P), ("HB2Tp", P),
                  ("HG1p", P), ("HG2p", P), ("HH2T", N), ("HH1T", N),
                  ("I91", N), ("DIAGF", N), ("MISC", 8)]:
    _B16[_name] = (_off, _w)
    _off += _w
_N16 = _off  # 1140


def _b16(name, w=None):
    o, bw = _B16[name]
    return slice(o, o + (w or bw))


class _FastTailTileContext(tile.TileContext):
    """TileContext with a single-shot epilogue: the NEFF runs exactly once
    per load here, so skip the semaphore/DMA-ring recycling and the second
    all-engine butterfly -- keep only the global drain (output-DMA safety)
    and a sequencer-level barrier."""

    def _drain_and_barrier(self, tick_clock, wait_clock):
        # No in-NEFF drain/barrier: NRT itself waits for engine-stream
        # completion and DMA-queue drain at execution end, and this NEFF is
        # never re-executed on a loaded core, so semaphore recycling is
        # unnecessary. The queue-drain wait costs ~7us of all-engine EVSEM
        # spinning when done inside the kernel.
        popped = self.nc._tile_sem_poison_stack.pop()
        assert popped is self._sem_poison


def build_nc() -> bacc.Bacc:
    nc = bacc.Bacc(None, target_bir_lowering=False, debug=False)
    hostf16 = nc.dram_tensor("hostf16", [N, _N16], F16, kind="ExternalInput")
    out = nc.dram_tensor("out", [1, 1], F32, kind="ExternalOutput")
    mult = mybir.AluOpType.mult

    with _FastTailTileContext(nc) as tc:
        with (
            tc.tile_pool(name="persist", bufs=1) as pp,
            tc.tile_pool(name="rot", bufs=3) as rot,
            tc.tile_pool(name="psv", bufs=3, space=bass.MemorySpace.PSUM) as psv,
            tc.tile_pool(name="psm", bufs=3, space=bass.MemorySpace.PSUM) as psm,
            tc.tile_pool(name="psr", bufs=1, space=bass.MemorySpace.PSUM) as psr,
        ):
            zb = pp.tile([N, 1], F32)
            nc.vector.memset(zb[:], 0.0)

            hm = pp.tile([N, _N16], F16)
            # Dgp/DgTp land first (own queue) so exp + Sinkhorn start early;
            # the bulk goes through SWDGE so its completion can't gate exp
            nc.sync.dma_start(hm[:, 0:2 * P], hostf16[:, 0:2 * P])
            nc.gpsimd.dma_start(hm[:, 2 * P:_N16], hostf16[:, 2 * P:_N16])
            mo = _B16["MISC"][0]
            wv = hm[0:1, mo:mo + 7]

            Rb = pp.tile([N, 1], F16)
            nc.vector.memset(Rb[:], 1.0)
            Cb = pp.tile([N, 1], F16)
            nc.vector.memset(Cb[:], 1.0)

            # [S0 | S0T] = exp(-0.5 * [Dgp | DgTp]), fp16, pad cols -> 0.
            # S0T half first: the first Sinkhorn matvec only needs s0t.
            eh = pp.tile([N, 2 * P], F16)
            nc.scalar.activation(eh[:, P:2 * P], hm[:, P:2 * P], EXP,
                                 bias=zb[:], scale=-0.5)
            nc.scalar.activation(eh[:, 0:P], hm[:, 0:P], EXP,
                                 bias=zb[:], scale=-0.5)
            s0 = eh[:, 0:P]
            s0t = eh[:, P:2 * P]

            # ---- Sinkhorn: R <- 1/(S0@C)[0:90], C <- 1/(S0'@R)[0:90] ----
            u = w = None
            with nc.allow_low_precision("fp16 pipeline; tol 2e-2"):
                for _ in range(SINKHORN_ITERS):
                    u = psv.tile([P, 1], F32, tag="vec")
                    nc.tensor.matmul(u[:], s0t, Cb[:])       # u = S0 @ C
                    nc.vector.reciprocal(Rb[0:NN, :], u[0:NN, :])
                    w = psv.tile([P, 1], F32, tag="vec")
                    nc.tensor.matmul(w[:], s0, Rb[:])        # w = S0' @ R
                    nc.vector.reciprocal(Cb[0:NN, :], w[0:NN, :])

                # fp32 copies of the final scales (off the critical path)
                Rf = pp.tile([N, 1], F32)
                nc.vector.memset(Rf[:], 1.0)
                nc.vector.reciprocal(Rf[0:NN, :], u[0:NN, :])
                Cf = pp.tile([N, 1], F32)
                nc.vector.memset(Cf[:], 1.0)
                nc.vector.reciprocal(Cf[0:NN, :], w[0:NN, :])

                cols = pp.tile([N, 7], F16)

                # T2: q0 = Vcol1 = C o w ; q = B2 @ q0 ; col1 = q0 o q
                q0 = pp.tile([N, 1], F16)
                nc.vector.tensor_scalar_mul(q0[:], w[0:N, :], Cf[:])
                q = psv.tile([P, 1], F32, tag="vec")
                nc.tensor.matmul(q[:], hm[:, _b16("HB2Tp")], q0[:])
                nc.vector.tensor_tensor(cols[:, 1:2], q[0:N, :], q0[:], mult)

                # T1: a = Vrow1 = R o (S0@C) ; p = B1 @ a ; col0 = a o p
                u2 = psv.tile([P, 1], F32, tag="vec")
                nc.tensor.matmul(u2[:], s0t, Cb[:])
                a = pp.tile([N, 1], F16)
                nc.vector.tensor_scalar_mul(a[:], u2[0:N, :], Rf[:])
                p = psv.tile([P, 1], F32, tag="vec")
                nc.tensor.matmul(p[:], hm[:, _b16("HB1p")], a[:])
                nc.vector.tensor_tensor(cols[:, 0:1], p[0:N, :], a[:], mult)

                # V = diag(R) S0 diag(C)   (via Bt = I)
                ic = rot.tile([N, N], F16, tag="bts")
                nc.vector.tensor_scalar_mul(ic[:], hm[:, _b16("I91")], Cf[:])
                pv = psm.tile([P, N], F32, tag="mat")
                nc.tensor.matmul(pv[:], s0t, ic[:])
                V = pp.tile([N, N], F32)
                nc.vector.tensor_scalar_mul(V[:], pv[0:N, :], Rf[:])

                # pair terms: col(2+i) = sum_k V o (L V Bt')
                pairs = [("HB1p", "HB2Tp"), ("HG1p", "HH2T"), ("HG2p", "HH1T")]
                for i, (lh, bt) in enumerate(pairs):
                    bo = _B16[bt][0]
                    bts = rot.tile([N, N], F16, tag="bts")
                    nc.vector.tensor_scalar_mul(bts[:], hm[:, bo:bo + N],
                                                Cf[:])
                    p1 = psm.tile([P, N], F32, tag="mat")
                    nc.tensor.matmul(p1[:], s0t, bts[:])   # S0 @ (diag(C) Bt)
                    y = rot.tile([N, N], F16, tag="y")
                    nc.vector.tensor_scalar_mul(y[:], p1[0:N, :], Rf[:])
                    z = psm.tile([P, N], F32, tag="mat")
                    nc.tensor.matmul(z[:], hm[:, _b16(lh)], y[:])  # L @ Y
                    jk = rot.tile([N, N], F16, tag="jk")
                    nc.vector.scalar_tensor_tensor(
                        jk[:], V[:], 1.0, z[0:N, :], op0=mult, op1=mult,
                        accum_out=cols[:, 2 + i:3 + i])

                # diag correction: col5 = sum_k diagF o V o V
                d2 = rot.tile([N, N], F32, tag="d2")
                nc.vector.tensor_tensor(d2[:], hm[:, _b16("DIAGF")], V[:],
                                        mult)
                jk = rot.tile([N, N], F16, tag="jk")
                nc.vector.scalar_tensor_tensor(
                    jk[:], d2[:], 1.0, V[:], op0=mult, op1=mult,
                    accum_out=cols[:, 5:6])
                # linear term: col6 = sum_k Dg o V
                jk = rot.tile([N, N], F16, tag="jk")
                nc.vector.scalar_tensor_tensor(
                    jk[:], hm[:, 0:N], 1.0, V[:], op0=mult, op1=mult,
                    accum_out=cols[:, 6:7])

                # fold partitions, weight, reduce to the ged scalar
                onesc = pp.tile([N, 1], F16)
                nc.vector.memset(onesc[:], 1.0)
                row = psr.tile([1, 7], F32)
                nc.tensor.matmul(row[:], onesc[:], cols[:])
                jr = pp.tile([1, 7], F32)
                g = pp.tile([1, 1], F32)
                nc.vector.scalar_tensor_tensor(
                    jr[:], row[:], 1.0, wv, op0=mult, op1=mult,
                    accum_out=g[:])
            nc.sync.dma_start(out[:], g[:])
    nc.compile()
    return nc


def host_prep(inputs: dict) -> np.ndarray:
    """Decode int adjacency/labels into the packed fp16 operand matrix."""
    node_weighs = np.asarray(inputs["node_weighs"], np.float32).reshape(-1)
    edge_weighs = np.asarray(inputs["edge_weighs"], np.float32).reshape(-1)
    cn = np.maximum(node_weighs, 0.0)
    ce = np.maximum(edge_weighs, 0.0)
    iu, ju = np.triu_indices(NB_LABELS, k=1)
    node_costs = np.zeros((NB_LABELS, NB_LABELS), np.float32)
    node_costs[iu, ju] = cn[:-1]
    node_costs = node_costs + node_costs.T
    nodeInsDel = np.float32(cn[-1])
    e = np.float32(ce[0])
    beta = np.float32(ce[-1])

    A = np.asarray(inputs["adjacenceMatrix"])
    A1 = np.zeros((N, N), np.int64)
    A1[:NN, :NN] = np.asarray(A[0][: NN * NN], np.int64).reshape(NN, NN)
    A2 = np.zeros((N, N), np.int64)
    A2[:NN, :NN] = np.asarray(A[1][: NN * NN], np.int64).reshape(NN, NN)
    Abin1 = (A1 != 0).astype(np.float16)
    Abin2 = (A2 != 0).astype(np.float16)
    G1 = (A1 == 1).astype(np.float16)
    G2 = (A1 == 2).astype(np.float16)
    H1 = (A2 == 1).astype(np.float16)
    H2 = (A2 == 2).astype(np.float16)

    labels = np.asarray(inputs["labels"])
    l1 = np.asarray(labels[0][:NN], np.int64)
    l2 = np.asarray(labels[1][:NN], np.int64)
    Dg = np.zeros((N, N), np.float32)
    Dg[:NN, :NN] = node_costs[l1[:, None], l2[None, :]]
    Dg[:NN, NN] = nodeInsDel
    Dg[NN, :NN] = nodeInsDel

    d1 = np.diag(Abin1).astype(np.float32)
    d2 = np.diag(Abin2).astype(np.float32)
    diagF = beta * (d1[:, None] + d2[None, :] - 2.0 * np.outer(d1, d2))
    diagF = diagF + e * (np.outer(np.diag(G1), np.diag(H2)).astype(np.float32)
                         + np.outer(np.diag(G2), np.diag(H1)).astype(np.float32))

    hostf16 = np.zeros((N, _N16), np.float16)

    def put(name, mat, pad=0.0):
        o, wd = _B16[name]
        hostf16[:, o:o + wd] = pad
        hostf16[:, o:o + mat.shape[1]] = mat

    put("Dgp", Dg.astype(np.float16), pad=80.0)      # exp(-0.5*80) -> 0
    put("DgTp", Dg.T.astype(np.float16), pad=80.0)
    put("HB1p", Abin1)
    put("HB2Tp", Abin2.T)
    put("HG1p", G1)
    put("HG2p", G2)
    put("HH2T", H2.T)
    put("HH1T", H1.T)
    put("I91", np.eye(N, dtype=np.float16))
    put("DIAGF", diagF.astype(np.float16))
    mo = _B16["MISC"][0]
    hostf16[0, mo:mo + 7] = np.array(
        [0.5 * beta, 0.5 * beta, -beta, 0.5 * e, 0.5 * e, -0.5, 1.0],
        np.float16)
    return hostf16


_NC_CACHE: list = []


def _run(inputs: dict, trace: bool = False):
    hostf16 = host_prep(inputs)
    if not _NC_CACHE:
        _NC_CACHE.append(build_nc())
    nc = _NC_CACHE[0]
    res = run_bass_kernel_spmd(
        nc, [{"hostf16": hostf16} for _ in range(N_CORES)],
        core_ids=list(range(N_CORES)), trace=trace)
    val = np.float32(np.asarray(res.results[0]["out"]).reshape(-1)[0])
    return val, res


def kernel(graph, adjacenceMatrix, graphCard, labels, node_weighs, edge_weighs):
    val, _ = _run({
        "adjacenceMatrix": adjacenceMatrix, "labels": labels,
        "node_weighs": node_weighs, "edge_weighs": edge_weighs,
    })
    return val


# revision 12
# speedup vs baseline: 1.0528x; 1.0017x over previous
"""GED layer (nn_GedLayer) on Trainium2 via Bass/Tile.

The reference builds an 8281x8281 cost matrix C = F + diag(c) where every
term of F is a Kronecker product of 91x91 matrices, runs a 10-iteration
Sinkhorn projection, and evaluates ged = 0.5*v'(F - diag(F))v + c'v.

Nothing 8281-sized is ever materialized here:
  * Sinkhorn on S = exp(-0.5*Dg) only rescales rows/cols, so we track the
    cumulative scale vectors:  R <- 1/(S0 @ C) (rows 0..89),
    C <- 1/(S0' @ R) (cols 0..89) -- two 91x91 matvecs per iteration on PE.
  * kron(P,Q) contractions reduce to 91x91 matmuls:
      sum V.(L V Rt') for (L,Rt) in {(B1,B2),(G1',H2),(G2',H1)}
    with V = diag(R) S0 diag(C), plus two rank-1 terms and the diagonal
    correction -- fused into per-partition accumulations + one ones-matvec.

Engine split: PE does matvecs/matmuls (fp16 operands, fp32 PSUM, 128-col
stationaries for fast weight load), DVE does the Sinkhorn reciprocals and
multiply-accumulate reductions, ACT does exp and all diag() row-scales
(activation Copy with a per-partition scale vector). Accuracy vs the fp32
reference: ~5e-4 rel, tolerance 2e-2. The host only decodes the int
adjacency/labels into the packed fp16 operand matrix (layout prep) and
shards it to the 8 cores; the problem is a single graph pair, so the
cores run the same tiny SPMD program and core 0's scalar is returned.
"""

import numpy as np

import concourse.bass as bass
import concourse.bacc as bacc
import concourse.mybir as mybir
import concourse.tile as tile
from concourse.bass_utils import run_bass_kernel_spmd

N = 91          # n+1 == m+1 with the epsilon node
NN = 90
P = 128         # padded stationary width (fast weight load needs 128 cols)
SINKHORN_ITERS = 10
NB_LABELS = 10
F16 = mybir.dt.float16
F32 = mybir.dt.float32
N_CORES = 8
COPY = mybir.ActivationFunctionType.Copy
EXP = mybir.ActivationFunctionType.Exp

# fp16 block layout: (name -> (col offset, width)). Stationary operands
# padded to 128 cols; pad cols of Dg/DgT hold 80.0 so exp(-0.5x) pads to 0.
_B16 = {}
_off = 0
for _name, _w in [("DgTp", P), ("Dgp", P), ("HB1p", P), ("HB2Tp", P),
                  ("HG1p", P), ("HG2p", P), ("HH2T", N), ("HH1T", N),
                  ("I91", N), ("DIAGF", N), ("MISC", 8)]:
    _B16[_name] = (_off, _w)
    _off += _w
_N16 = _off  # 1140


def _b16(name, w=None):
    o, bw = _B16[name]
    return slice(o, o + (w or bw))


class _FastTailTileContext(tile.TileContext):
    """TileContext with a single-shot epilogue: the NEFF runs exactly once
    per load here, so skip the semaphore/DMA-ring recycling and the second
    all-engine butterfly -- keep only the global drain (output-DMA safety)
    and a sequencer-level barrier."""

    def _drain_and_barrier(self, tick_clock, wait_clock):
        # No in-NEFF drain/barrier: NRT itself waits for engine-stream
        # completion and DMA-queue drain at execution end, and this NEFF is
        # never re-executed on a loaded core, so semaphore recycling is
        # unnecessary. The queue-drain wait costs ~7us of all-engine EVSEM
        # spinning when done inside the kernel.
        popped = self.nc._tile_sem_poison_stack.pop()
        assert popped is self._sem_poison


def build_nc() -> bacc.Bacc:
    nc = bacc.Bacc(None, target_bir_lowering=False, debug=False)
    hostf16 = nc.dram_tensor("hostf16", [N, _N16], F16, kind="ExternalInput")
    out = nc.dram_tensor("out", [1, 1], F32, kind="ExternalOutput")
    mult = mybir.AluOpType.mult

    with _FastTailTileContext(nc) as tc:
        with (
            tc.tile_pool(name="persist", bufs=1) as pp,
            tc.tile_pool(name="rot", bufs=4) as rot,
            tc.tile_pool(name="psv", bufs=3, space=bass.MemorySpace.PSUM) as psv,
            tc.tile_pool(name="psm", bufs=3, space=bass.MemorySpace.PSUM) as psm,
            tc.tile_pool(name="psr", bufs=1, space=bass.MemorySpace.PSUM) as psr,
        ):
            zb = pp.tile([N, 1], F32)
            nc.vector.memset(zb[:], 0.0)
            warm = pp.tile([1, 1], F32)
            # prewarm the ACT Exp table while the DMAs are in flight
            nc.scalar.activation(warm[:], zb[0:1, :], EXP,
                                 bias=zb[0:1, :], scale=1.0)

            hm = pp.tile([N, _N16], F16)
            # DgTp lands first on its own queue: the first Sinkhorn matvec
            # needs only S0T. Everything else goes through SWDGE so its
            # completion can't gate the exp.
            nc.sync.dma_start(hm[:, 0:P], hostf16[:, 0:P])
            nc.gpsimd.dma_start(hm[:, P:_N16], hostf16[:, P:_N16])
            mo = _B16["MISC"][0]
            wv = hm[0:1, mo:mo + 7]

            Rb = pp.tile([N, 1], F16)
            nc.vector.memset(Rb[:], 1.0)
            Cb = pp.tile([N, 1], F16)
            nc.vector.memset(Cb[:], 1.0)

            # [S0T | S0] = exp(-0.5 * [DgTp | Dgp]), fp16, pad cols -> 0.
            # S0T half first: the first Sinkhorn matvec only needs s0t.
            eh = pp.tile([N, 2 * P], F16)
            nc.scalar.activation(eh[:, 0:P], hm[:, 0:P], EXP,
                                 bias=zb[:], scale=-0.5)
            nc.scalar.activation(eh[:, P:2 * P], hm[:, P:2 * P], EXP,
                                 bias=zb[:], scale=-0.5)
            s0t = eh[:, 0:P]
            s0 = eh[:, P:2 * P]

            # ---- Sinkhorn: R <- 1/(S0@C)[0:90], C <- 1/(S0'@R)[0:90] ----
            u = w = None
            with nc.allow_low_precision("fp16 pipeline; tol 2e-2"):
                for _ in range(SINKHORN_ITERS):
                    u = psv.tile([P, 1], F32, tag="vec")
                    nc.tensor.matmul(u[:], s0t, Cb[:])       # u = S0 @ C
                    nc.vector.reciprocal(Rb[0:NN, :], u[0:NN, :])
                    w = psv.tile([P, 1], F32, tag="vec")
                    nc.tensor.matmul(w[:], s0, Rb[:])        # w = S0' @ R
                    nc.vector.reciprocal(Cb[0:NN, :], w[0:NN, :])

                # fp32 copies of the final scales (off the critical path)
                Rf = pp.tile([N, 1], F32)
                nc.vector.memset(Rf[:], 1.0)
                nc.vector.reciprocal(Rf[0:NN, :], u[0:NN, :])
                Cf = pp.tile([N, 1], F32)
                nc.vector.memset(Cf[:], 1.0)
                nc.vector.reciprocal(Cf[0:NN, :], w[0:NN, :])

                cols = pp.tile([N, 7], F16)

                # T2: q0 = Vcol1 = C o w ; q = B2 @ q0 ; col1 = q0 o q
                q0 = pp.tile([N, 1], F16)
                nc.vector.tensor_scalar_mul(q0[:], w[0:N, :], Cf[:])
                q = psv.tile([P, 1], F32, tag="vec")
                nc.tensor.matmul(q[:], hm[:, _b16("HB2Tp")], q0[:])
                nc.vector.tensor_tensor(cols[:, 1:2], q[0:N, :], q0[:], mult)

                # T1: a = Vrow1 = R o (S0@C) ; p = B1 @ a ; col0 = a o p
                u2 = psv.tile([P, 1], F32, tag="vec")
                nc.tensor.matmul(u2[:], s0t, Cb[:])
                a = pp.tile([N, 1], F16)
                nc.vector.tensor_scalar_mul(a[:], u2[0:N, :], Rf[:])
                p = psv.tile([P, 1], F32, tag="vec")
                nc.tensor.matmul(p[:], hm[:, _b16("HB1p")], a[:])
                nc.vector.tensor_tensor(cols[:, 0:1], p[0:N, :], a[:], mult)

                # V = diag(R) S0 diag(C)   (via Bt = I)
                ic = rot.tile([N, N], F16, tag="bts")
                nc.vector.tensor_scalar_mul(ic[:], hm[:, _b16("I91")], Cf[:])
                pv = psm.tile([P, N], F32, tag="mat")
                nc.tensor.matmul(pv[:], s0t, ic[:])
                V = pp.tile([N, N], F32)
                nc.vector.tensor_scalar_mul(V[:], pv[0:N, :], Rf[:])

                # pair terms: col(2+i) = sum_k V o (L V Bt')
                pairs = [("HB1p", "HB2Tp"), ("HG1p", "HH2T"), ("HG2p", "HH1T")]
                for i, (lh, bt) in enumerate(pairs):
                    bo = _B16[bt][0]
                    bts = rot.tile([N, N], F16, tag="bts")
                    nc.vector.tensor_scalar_mul(bts[:], hm[:, bo:bo + N],
                                                Cf[:])
                    p1 = psm.tile([P, N], F32, tag="mat")
                    nc.tensor.matmul(p1[:], s0t, bts[:])   # S0 @ (diag(C) Bt)
                    y = rot.tile([N, N], F16, tag="y")
                    nc.vector.tensor_scalar_mul(y[:], p1[0:N, :], Rf[:])
                    z = psm.tile([P, N], F32, tag="mat")
                    nc.tensor.matmul(z[:], hm[:, _b16(lh)], y[:])  # L @ Y
                    jk = rot.tile([N, N], F16, tag="jk")
                    nc.vector.scalar_tensor_tensor(
                        jk[:], V[:], 1.0, z[0:N, :], op0=mult, op1=mult,
                        accum_out=cols[:, 2 + i:3 + i])

                # diag correction: col5 = sum_k diagF o V o V
                d2 = rot.tile([N, N], F32, tag="d2")
                nc.vector.tensor_tensor(d2[:], hm[:, _b16("DIAGF")], V[:],
                                        mult)
                jk = rot.tile([N, N], F16, tag="jk")
                nc.vector.scalar_tensor_tensor(
                    jk[:], d2[:], 1.0, V[:], op0=mult, op1=mult,
                    accum_out=cols[:, 5:6])
                # linear term: col6 = sum_k Dg o V
                jk = rot.tile([N, N], F16, tag="jk")
                dgo = _B16["Dgp"][0]
                nc.vector.scalar_tensor_tensor(
                    jk[:], hm[:, dgo:dgo + N], 1.0, V[:], op0=mult, op1=mult,
                    accum_out=cols[:, 6:7])

                # fold partitions, weight, reduce to the ged scalar
                onesc = pp.tile([N, 1], F16)
                nc.vector.memset(onesc[:], 1.0)
                row = psr.tile([1, 7], F32)
                nc.tensor.matmul(row[:], onesc[:], cols[:])
                jr = pp.tile([1, 7], F32)
                g = pp.tile([1, 1], F32)
                nc.vector.scalar_tensor_tensor(
                    jr[:], row[:], 1.0, wv, op0=mult, op1=mult,
                    accum_out=g[:])
            nc.sync.dma_start(out[:], g[:])
    nc.compile()
    return nc


def host_prep(inputs: dict) -> np.ndarray:
    """Decode int adjacency/labels into the packed fp16 operand matrix."""
    node_weighs = np.asarray(inputs["node_weighs"], np.float32).reshape(-1)
    edge_weighs = np.asarray(inputs["edge_weighs"], np.float32).reshape(-1)
    cn = np.maximum(node_weighs, 0.0)
    ce = np.maximum(edge_weighs, 0.0)
    iu, ju = np.triu_indices(NB_LABELS, k=1)
    node_costs = np.zeros((NB_LABELS, NB_LABELS), np.float32)
    node_costs[iu, ju] = cn[:-1]
    node_costs = node_costs + node_costs.T
    nodeInsDel = np.float32(cn[-1])
    e = np.float32(ce[0])
    beta = np.float32(ce[-1])

    A = np.asarray(inputs["adjacenceMatrix"])
    A1 = np.zeros((N, N), np.int64)
    A1[:NN, :NN] = np.asarray(A[0][: NN * NN], np.int64).reshape(NN, NN)
    A2 = np.zeros((N, N), np.int64)
    A2[:NN, :NN] = np.asarray(A[1][: NN * NN], np.int64).reshape(NN, NN)
    Abin1 = (A1 != 0).astype(np.float16)
    Abin2 = (A2 != 0).astype(np.float16)
    G1 = (A1 == 1).astype(np.float16)
    G2 = (A1 == 2).astype(np.float16)
    H1 = (A2 == 1).astype(np.float16)
    H2 = (A2 == 2).astype(np.float16)

    labels = np.asarray(inputs["labels"])
    l1 = np.asarray(labels[0][:NN], np.int64)
    l2 = np.asarray(labels[1][:NN], np.int64)
    Dg = np.zeros((N, N), np.float32)
    Dg[:NN, :NN] = node_costs[l1[:, None], l2[None, :]]
    Dg[:NN, NN] = nodeInsDel
    Dg[NN, :NN] = nodeInsDel

    d1 = np.diag(Abin1).astype(np.float32)
    d2 = np.diag(Abin2).astype(np.float32)
    diagF = beta * (d1[:, None] + d2[None, :] - 2.0 * np.outer(d1, d2))
    diagF = diagF + e * (np.outer(np.diag(G1), np.diag(H2)).astype(np.float32)
                         + np.outer(np.diag(G2), np.diag(H1)).astype(np.float32))

    hostf16 = np.zeros((N, _N16), np.float16)

    def put(name, mat, pad=0.0):
        o, wd = _B16[name]
        hostf16[:, o:o + wd] = pad
        hostf16[:, o:o + mat.shape[1]] = mat

    put("Dgp", Dg.astype(np.float16), pad=80.0)      # exp(-0.5*80) -> 0
    put("DgTp", Dg.T.astype(np.float16), pad=80.0)
    put("HB1p", Abin1)
    put("HB2Tp", Abin2.T)
    put("HG1p", G1)
    put("HG2p", G2)
    put("HH2T", H2.T)
    put("HH1T", H1.T)
    put("I91", np.eye(N, dtype=np.float16))
    put("DIAGF", diagF.astype(np.float16))
    mo = _B16["MISC"][0]
    hostf16[0, mo:mo + 7] = np.array(
        [0.5 * beta, 0.5 * beta, -beta, 0.5 * e, 0.5 * e, -0.5, 1.0],
        np.float16)
    return hostf16


_NC_CACHE: list = []


def _run(inputs: dict, trace: bool = False):
    hostf16 = host_prep(inputs)
    if not _NC_CACHE:
        _NC_CACHE.append(build_nc())
    nc = _NC_CACHE[0]
    res = run_bass_kernel_spmd(
        nc, [{"hostf16": hostf16} for _ in range(N_CORES)],
        core_ids=list(range(N_CORES)), trace=trace)
    val = np.float32(np.asarray(res.results[0]["out"]).reshape(-1)[0])
    return val, res


def kernel(graph, adjacenceMatrix, graphCard, labels, node_weighs, edge_weighs):
    val, _ = _run({
        "adjacenceMatrix": adjacenceMatrix, "labels": labels,
        "node_weighs": node_weighs, "edge_weighs": edge_weighs,
    })
    return val
